# revision 1
# baseline (speedup 1.0000x reference)
"""LayerNorm-LSTM (2-layer, shared h/c across layers, per-sample weights) on 8 TRN2 cores.

Strategy: data-parallel over batch (2 samples/core). Per core:
  Phase A: load inputs; Phase B: precompute layer-0 input projections
  gates_x[t] = Wih0 @ x_t + b0 for all t (batched matmul, N=256);
  Phase C: sequential recurrence over t. The h-recurrence matvecs run in
  moving-weight mode (stationary = h K-block [128,1], moving = W^T chunk
  [128,512] viewed float32r -> 1 cycle/row). Gate rows [4,512] are PE-
  transposed to column layout [128,16] (pos = cc*128 + p) so LayerNorm
  stats come from a ones-matmul partition reduction and all elementwise
  work runs at small free-dims across 128 partitions. h is produced
  directly in column layout and feeds the next matvec's stationary.

Gate order is host-permuted from [i,f,g,o] to [i,f,o,g] so one sigmoid
covers cols 0..11 and one tanh covers 12..15.
"""

import sys

sys.path.insert(0, "/opt/trn_rl_repo")

import numpy as np

import concourse.bacc as bacc
import concourse.bass as bass
import concourse.tile as tile
from concourse import mybir
from concourse.bass_utils import run_bass_kernel_spmd

F32 = mybir.dt.float32
F32R = mybir.dt.float32r
BF16 = mybir.dt.bfloat16
AF = mybir.ActivationFunctionType
OP = mybir.AluOpType

B, S, D, H = 16, 256, 512, 512
NCORES = 8
BPC = B // NCORES  # samples per core
EPS = 1e-5
# permutation taking reference gate order [i,f,g,o] -> [i,f,o,g]
GATE_PERM = np.concatenate(
    [np.arange(0, 512), np.arange(512, 1024), np.arange(1536, 2048), np.arange(1024, 1536)]
)


def build_program(s_steps=S, apply_ln_affine=False):
    """Build the per-core SPMD Bass program. Returns the compiled Bacc."""
    nc = bacc.Bacc("TRN2", target_bir_lowering=False, debug=False, num_devices=NCORES)

    # ---- DRAM parameters (per-core shapes) ----
    # xT[s, k, d, t] = x[s, t, 128k+d]
    xT_d = nc.dram_tensor("xT", [BPC, 4, 128, s_steps], F32, kind="ExternalInput").ap()
    # w0T[s, m, k, d, j] = W0_perm[128m+j, 128k+d]  (x-proj stationary tiles)
    w0T_d = nc.dram_tensor("w0T", [BPC, 16, 4, 128, 128], F32, kind="ExternalInput").ap()
    # whT[s, l, k, d, g] = Wrec_perm[g, 128k+d] ; l=0: whh0, l=1: wih1+whh1
    whT_d = nc.dram_tensor("whT", [BPC, 2, 4, 128, 2048], F32R, kind="ExternalInput").ap()
    # bias cols: b0col[p, s, m] = b0_perm[128m+p]; b1col same for layer 1
    b0c_d = nc.dram_tensor("b0col", [128, BPC, 16], F32, kind="ExternalInput").ap()
    b1c_d = nc.dram_tensor("b1col", [128, BPC, 16], F32, kind="ExternalInput").ap()
    # LN affine replicated to column layout (only used if apply_ln_affine)
    lnw_d = nc.dram_tensor("lnw", [128, 2, 16], F32, kind="ExternalInput").ap()
    lnb_d = nc.dram_tensor("lnb", [128, 2, 16], F32, kind="ExternalInput").ap()
    # output: ys[p, s, t, cc] = h1(t)[cc*128+p] for sample s
    ys_d = nc.dram_tensor("ys", [128, BPC, s_steps, 4], F32, kind="ExternalOutput").ap()

    from contextlib import ExitStack

    with tile.TileContext(nc) as tc, ExitStack() as ctx:
        consts = ctx.enter_context(tc.tile_pool(name="consts", bufs=1))
        wpool = ctx.enter_context(tc.tile_pool(name="weights", bufs=1))
        xpool = ctx.enter_context(tc.tile_pool(name="xproj", bufs=6))
        state = ctx.enter_context(tc.tile_pool(name="state", bufs=1))
        work = ctx.enter_context(tc.tile_pool(name="work", bufs=3))
        h0pool = ctx.enter_context(tc.tile_pool(name="h0", bufs=2))
        psg = ctx.enter_context(tc.tile_pool(name="psg", bufs=3, space="PSUM"))
        psum = ctx.enter_context(tc.tile_pool(name="psum", bufs=2, space="PSUM"))

        if True:
            # ---- constants / persistent tiles ----
            ident = consts.tile([4, 4], F32)
            nc.gpsimd.memset(ident, 0.0)
            from concourse.masks import make_identity

            make_identity(nc, ident, nomemset=True)

            ones = consts.tile([128, 128], F32)
            nc.vector.memset(ones, 1.0)
            epsb = consts.tile([128, 1], F32)
            nc.vector.memset(epsb, EPS)
            hzero_f = consts.tile([128, 4], F32)
            nc.vector.memset(hzero_f, 0.0)
            hzero = consts.tile([128, 4], F32R)
            nc.vector.tensor_copy(hzero, hzero_f)

            b0col = consts.tile([128, BPC, 16], F32)
            nc.sync.dma_start(out=b0col, in_=b0c_d)
            b1col = consts.tile([128, BPC, 16], F32)
            nc.sync.dma_start(out=b1col, in_=b1c_d)
            if apply_ln_affine:
                lnw = consts.tile([128, 2, 16], F32)
                nc.sync.dma_start(out=lnw, in_=lnw_d)
                lnb = consts.tile([128, 2, 16], F32)
                nc.sync.dma_start(out=lnb, in_=lnb_d)

            # recurrent weights, SBUF-resident (16 MB), partition-first
            whT = wpool.tile([128, BPC, 2, 4, 2048], F32R)
            for s in range(BPC):
                for l in range(2):
                    for k in range(4):
                        nc.sync.dma_start(out=whT[:, s, l, k], in_=whT_d[s, l, k])

            # gates_x in col layout, bf16: gx[p, s, t, m]
            gx = wpool.tile([128, BPC, s_steps, 16], F32)
            # xT resident (1 MB), partition-first
            xTs = wpool.tile([128, BPC, 4, s_steps], F32)
            for s in range(BPC):
                for k in range(4):
                    nc.sync.dma_start(out=xTs[:, s, k], in_=xT_d[s, k])

            # ---- Phase B: x-projection ----
            for s in range(BPC):
                for m in range(16):
                    pxa = psum.tile([128, s_steps], F32, tag="sums")
                    wt = []
                    for _k in range(4):
                        w0t = xpool.tile([128, 128], F32, tag="w0t")
                        wt.append(w0t)
                    for k in range(4):
                        nc.sync.dma_start(out=wt[k], in_=w0T_d[s, m, k])
                    for k in range(4):
                        nc.tensor.matmul(
                            pxa,
                            wt[k],
                            xTs[:, s, k],
                            start=(k == 0),
                            stop=(k == 3),
                        )
                    # gx[:, s, :, m] = pxa + b0col[:, s, m]
                    nc.vector.tensor_scalar(
                        gx[:, s, :, m : m + 1].rearrange("p t o -> p (t o)"),
                        pxa,
                        b0col[:, s, m : m + 1],
                        None,
                        OP.add,
                    )

            # ---- persistent recurrence state ----
            # ys accumulates h1 history in SBUF; also serves as L0's h input
            ys_sb = state.tile([128, BPC, s_steps, 4], F32R)
            # cstate[:, s, 0:4] = c, [:, s, 4:8] = c^2
            cst = state.tile([128, BPC, 8], F32)
            nc.vector.memset(cst, 0.0)

            # ---- Phase C: recurrence ----
            for t in range(s_steps):
                h0t = h0pool.tile([128, BPC, 4], F32R, tag="h0")
                for l in range(2):
                    gcols = {}
                    for s in range(BPC):
                        # --- matvec: gates = Wrec @ h ---
                        if l == 0:
                            hin = (
                                hzero
                                if t == 0
                                else ys_sb[:, s, t - 1]
                            )  # [128, 4]
                        else:
                            hin = h0t[:, s]
                        psgA = psg.tile([1, 1024], F32, tag="g")
                        psgB = psg.tile([1, 1024], F32, tag="g")
                        halves = (psgA, psgB)
                        # half A fully accumulated first so its evac+scatter
                        # overlaps half B's matmuls
                        for hf in range(2):
                            for k in range(4):
                                lhs = hin[:, k : k + 1].bitcast(F32R)
                                for j in range(2):
                                    ch = hf * 2 + j
                                    nc.tensor.matmul(
                                        halves[hf][0:1, j * 512 : (j + 1) * 512],
                                        lhs,
                                        whT[:, s, l, k, ch * 512 : (ch + 1) * 512].bitcast(F32R),
                                        start=(k == 0),
                                        stop=(k == 3),
                                    )
                        # --- evacuate row to SBUF (DVE+ACT halves), then ONE
                        # scatter DMA [1,2048] -> [128,16]; whT columns are
                        # host-interleaved so psum col n = gcol(n//16, n%16) ---
                        grow = work.tile([1, 2048], F32, tag="grow")
                        gcol = work.tile([128, 16], F32, tag="gcol")
                        nc.vector.tensor_copy(grow[0:1, 0:1024], psgA)
                        nc.sync.dma_start(out=gcol[0:64, :], in_=grow[0:1, 0:1024])
                        nc.scalar.copy(grow[0:1, 1024:2048], psgB)
                        nc.sync.dma_start(out=gcol[64:128, :], in_=grow[0:1, 1024:2048])
                        gcols[s] = gcol
                    for s in range(BPC):
                        gcol = gcols[s]
                        # --- combo = [gates+bias | (gates+bias)^2 ] ---
                        combo = work.tile([128, 32], F32, tag="combo")
                        gsb = combo[:, 0:16].rearrange("p (g cc) -> p g cc", g=4)
                        badd_in1 = (
                            gx[:, s, t] if l == 0 else b1col[:, s]
                        )  # [128,16] m-conv (g-major)
                        for pa, pb in ((0, 64), (64, 128)):
                            nc.vector.tensor_tensor(
                                combo[pa:pb, 0:16], gcol[pa:pb], badd_in1[pa:pb], OP.add
                            )
                            nc.vector.tensor_tensor(
                                combo[pa:pb, 16:32],
                                combo[pa:pb, 0:16],
                                combo[pa:pb, 0:16],
                                OP.mult,
                            )
                        # --- LN stats via ones-matmul partition reduction ---
                        psums = psum.tile([128, 32], F32, tag="sums")
                        nc.tensor.matmul(psums, ones, combo, start=True, stop=True)
                        E = work.tile([128, 32], F32, tag="E")
                        nc.vector.tensor_scalar(E, psums, 1.0 / 512.0, None, OP.mult)
                        # fold over cc in one reduce: E viewed [p, (h g) 8, cc 4]
                        St = work.tile([128, 8], F32, tag="St")  # [ (h g) ]
                        nc.vector.tensor_reduce(
                            St,
                            E.rearrange("p (h g cc) -> p (h g) cc", h=2, cc=4),
                            mybir.AxisListType.X,
                            OP.add,
                        )
                        mean = St[:, 0:4]  # per gate
                        ex2 = St[:, 4:8]
                        var = work.tile([128, 4], F32, tag="var")
                        nc.vector.tensor_tensor(var, mean, mean, OP.mult)
                        nc.vector.tensor_tensor(var, ex2, var, OP.subtract)
                        rstd = work.tile([128, 4], F32, tag="rstd")
                        nc.scalar.activation(rstd, var, AF.Sqrt, bias=epsb, scale=1.0)
                        nc.vector.reciprocal(rstd, rstd)
                        # --- normalize, (optional affine), activations ---
                        wk = work.tile([128, 16], F32, tag="wk")
                        wkg = wk.rearrange("p (g cc) -> p g cc", g=4)
                        nc.vector.tensor_tensor(
                            wkg,
                            gsb,
                            mean[:, :, None].to_broadcast((128, 4, 4)),
                            OP.subtract,
                        )
                        nc.vector.tensor_tensor(
                            wkg,
                            wkg,
                            rstd[:, :, None].to_broadcast((128, 4, 4)),
                            OP.mult,
                        )
                        if apply_ln_affine:
                            nc.vector.tensor_tensor(wk, wk, lnw[:, l], OP.mult)
                            nc.vector.tensor_tensor(wk, wk, lnb[:, l], OP.add)
                        nc.scalar.activation(wk[:, 0:12], wk[:, 0:12], AF.Sigmoid)
                        nc.scalar.activation(wk[:, 12:16], wk[:, 12:16], AF.Tanh)
                        # --- c update: c = f*c + i*g ---
                        tmp = work.tile([128, 8], F32, tag="tmp")
                        nc.vector.tensor_tensor(
                            tmp[:, 0:4], wk[:, 0:4], wk[:, 12:16], OP.mult
                        )  # i*g
                        nc.vector.tensor_tensor(
                            tmp[:, 4:8], wk[:, 4:8], cst[:, s, 0:4], OP.mult
                        )  # f*c
                        nc.vector.tensor_tensor(
                            cst[:, s, 0:4], tmp[:, 0:4], tmp[:, 4:8], OP.add
                        )
                        nc.vector.tensor_tensor(
                            cst[:, s, 4:8], cst[:, s, 0:4], cst[:, s, 0:4], OP.mult
                        )
                        # --- LN(c) ---
                        pcs_full = psum.tile([128, 32], F32, tag="sums")
                        pcs = pcs_full[:, 0:8]
                        nc.tensor.matmul(pcs, ones, cst[:, s], start=True, stop=True)
                        CE = work.tile([128, 8], F32, tag="CE")
                        nc.vector.tensor_scalar(CE, pcs, 1.0 / 512.0, None, OP.mult)
                        CS = work.tile([128, 2], F32, tag="CS")
                        nc.vector.tensor_reduce(
                            CS,
                            CE.rearrange("p (h cc) -> p h cc", h=2),
                            mybir.AxisListType.X,
                            OP.add,
                        )
                        cvar = work.tile([128, 1], F32, tag="cvar")
                        nc.vector.tensor_tensor(cvar, CS[:, 0:1], CS[:, 0:1], OP.mult)
                        nc.vector.tensor_tensor(cvar, CS[:, 1:2], cvar, OP.subtract)
                        crstd = work.tile([128, 1], F32, tag="crstd")
                        nc.scalar.activation(crstd, cvar, AF.Sqrt, bias=epsb, scale=1.0)
                        nc.vector.reciprocal(crstd, crstd)
                        lnc = work.tile([128, 4], F32, tag="lnc")
                        nc.vector.tensor_tensor(
                            lnc,
                            cst[:, s, 0:4],
                            CS[:, 0:1].to_broadcast((128, 4)),
                            OP.subtract,
                        )
                        nc.vector.tensor_tensor(
                            lnc, lnc, crstd.to_broadcast((128, 4)), OP.mult
                        )
                        if apply_ln_affine:
                            nc.vector.tensor_tensor(
                                lnc, lnc, lnw[:, l, 0:4], OP.mult
                            )
                            nc.vector.tensor_tensor(lnc, lnc, lnb[:, l, 0:4], OP.add)
                        nc.scalar.activation(lnc, lnc, AF.Tanh)
                        # --- h = o * tanh(ln(c)) ---
                        hdst = h0t[:, s] if l == 0 else ys_sb[:, s, t]
                        nc.vector.tensor_tensor(hdst, wk[:, 8:12], lnc, OP.mult)

            # ---- output DMA ----
            for s in range(BPC):
                nc.sync.dma_start(out=ys_d[:, s], in_=ys_sb[:, s].bitcast(F32))

    nc.compile()
    return nc


_CACHE = {}


def _get_program(s_steps=S, affine=False):
    key = (s_steps, affine)
    if key not in _CACHE:
        _CACHE[key] = build_program(s_steps, apply_ln_affine=affine)
    return _CACHE[key]


def make_in_maps(x, wih0, whh0, bih0, bhh0, wih1, whh1, bih1, bhh1, ln_w, ln_b, s_steps=S):
    """Host-side preprocessing: shard + reformat inputs for the 8 cores."""
    x = np.asarray(x, np.float32)[:, :s_steps]
    perm = GATE_PERM
    in_maps = []
    for c in range(NCORES):
        sl = slice(c * BPC, (c + 1) * BPC)
        xs = x[sl]  # [BPC, s, 512]
        w0p = np.asarray(wih0, np.float32)[sl][:, perm]  # [BPC, 2048, 512]
        wh0p = np.asarray(whh0, np.float32)[sl][:, perm]
        w1p = (np.asarray(wih1, np.float32) + np.asarray(whh1, np.float32))[sl][:, perm]
        b0p = (np.asarray(bih0, np.float32) + np.asarray(bhh0, np.float32))[sl][:, perm]
        b1p = (np.asarray(bih1, np.float32) + np.asarray(bhh1, np.float32))[sl][:, perm]

        # position convention: vector index pos maps to (p = pos//4, cc = pos%4);
        # contraction block k = residue: h-tile column k holds h[4p + k]
        # xT[s, k, d', t] = x[s, t, 4d'+k]
        xT = np.ascontiguousarray(
            xs.transpose(0, 2, 1).reshape(BPC, 128, 4, s_steps).transpose(0, 2, 1, 3)
        )
        # w0T[s, m=(g,q), k, d', j] = W0_perm[512g + 4j + q, 4d' + k]
        w0v = w0p.reshape(BPC, 4, 128, 4, 128, 4)  # [s, g, j, q, d', k]
        w0T = np.ascontiguousarray(w0v.transpose(0, 1, 3, 5, 4, 2).reshape(BPC, 16, 4, 128, 128))
        # whT[s, l, k, d', n] = Wrec_perm[r(n), 4d' + k] with the output
        # rows interleaved so psum col n lands at gcol(p=n//16, m=n%16)
        n_idx = np.arange(2048)
        r_idx = 512 * ((n_idx % 16) // 4) + 4 * (n_idx // 16) + (n_idx % 4)
        whT = np.stack([wh0p, w1p], axis=1)[:, :, r_idx]  # [BPC, 2, 2048, 512]
        whT = np.ascontiguousarray(
            whT.reshape(BPC, 2, 2048, 128, 4).transpose(0, 1, 4, 3, 2)
        )
        # b0col[p, s, g*4+q] = b0_perm[512g + 4p + q]
        b0col = np.ascontiguousarray(
            b0p.reshape(BPC, 4, 128, 4).transpose(2, 0, 1, 3).reshape(128, BPC, 16)
        )
        b1col = np.ascontiguousarray(
            b1p.reshape(BPC, 4, 128, 4).transpose(2, 0, 1, 3).reshape(128, BPC, 16)
        )
        # ln affine replicated (only consumed if apply_ln_affine)
        lnw_rep = np.ascontiguousarray(
            np.broadcast_to(
                np.asarray(ln_w, np.float32).reshape(2, 128, 4).transpose(1, 0, 2)[:, :, None, :],
                (128, 2, 4, 4),
            ).reshape(128, 2, 16)
        )
        lnb_rep = np.ascontiguousarray(
            np.broadcast_to(
                np.asarray(ln_b, np.float32).reshape(2, 128, 4).transpose(1, 0, 2)[:, :, None, :],
                (128, 2, 4, 4),
            ).reshape(128, 2, 16)
        )
        in_maps.append(
            {
                "xT": xT,
                "w0T": w0T,
                "whT": whT,
                "b0col": b0col,
                "b1col": b1col,
                "lnw": lnw_rep,
                "lnb": lnb_rep,
            }
        )
    return in_maps


def assemble_output(results, s_steps=S):
    ys = np.empty((B, s_steps, H), np.float32)
    for c in range(NCORES):
        out = results[c]["ys"]  # [128, BPC, s, 4]
        for s in range(BPC):
            # ys[b, t, 4p+cc] = out[p, s, t, cc]
            ys[c * BPC + s] = out[:, s].transpose(1, 0, 2).reshape(s_steps, H)
    return ys


def kernel(**inputs):
    s_steps = S
    affine = not (
        np.all(np.asarray(inputs["ln_w"]) == 1.0)
        and np.all(np.asarray(inputs["ln_b"]) == 0.0)
    )
    nc = _get_program(s_steps, affine)
    in_maps = make_in_maps(**inputs, s_steps=s_steps)
    res = run_bass_kernel_spmd(nc, in_maps, list(range(NCORES)))
    return assemble_output(res.results, s_steps)


if __name__ == "__main__":
    # quick small-S self-test against a numpy reference
    s_steps = int(sys.argv[1]) if len(sys.argv) > 1 else 8

    rng = np.random.default_rng(0)
    WS = 0.02
    inputs = {
        "x": rng.standard_normal((B, S, D), np.float32),
        "wih0": rng.standard_normal((B, 2048, D), np.float32) * WS,
        "whh0": rng.standard_normal((B, 2048, H), np.float32) * WS,
        "bih0": rng.standard_normal((B, 2048), np.float32) * WS,
        "bhh0": rng.standard_normal((B, 2048), np.float32) * WS,
        "wih1": rng.standard_normal((B, 2048, H), np.float32) * WS,
        "whh1": rng.standard_normal((B, 2048, H), np.float32) * WS,
        "bih1": rng.standard_normal((B, 2048), np.float32) * WS,
        "bhh1": rng.standard_normal((B, 2048), np.float32) * WS,
        "ln_w": np.ones((2, H), np.float32),
        "ln_b": np.zeros((2, H), np.float32),
    }

    def np_ref(inputs, s_steps):
        def ln(v):
            m = v.mean(-1, keepdims=True)
            va = ((v - m) ** 2).mean(-1, keepdims=True)
            return (v - m) / np.sqrt(va + EPS)

        def sig(v):
            return 1.0 / (1.0 + np.exp(-v))

        x = inputs["x"][:, :s_steps].astype(np.float64)
        h = np.zeros((B, H))
        c = np.zeros((B, H))
        ys = np.zeros((B, s_steps, H))
        for t in range(s_steps):
            cur = x[:, t]
            for l, (wi, wh, bi, bh) in enumerate(
                [
                    (inputs["wih0"], inputs["whh0"], inputs["bih0"], inputs["bhh0"]),
                    (inputs["wih1"], inputs["whh1"], inputs["bih1"], inputs["bhh1"]),
                ]
            ):
                gates = (
                    np.einsum("bgd,bd->bg", wi.astype(np.float64), cur)
                    + np.einsum("bgh,bh->bg", wh.astype(np.float64), h)
                    + bi
                    + bh
                )
                i, f, g, o = np.split(gates, 4, axis=1)
                i, f, g, o = sig(ln(i)), sig(ln(f)), np.tanh(ln(g)), sig(ln(o))
                c = f * c + i * g
                h = o * np.tanh(ln(c))
                cur = h
            ys[:, t] = h
        return ys

    import time

    t0 = time.time()
    nc = build_program(s_steps)
    print(f"build+schedule+compile: {time.time()-t0:.1f}s", flush=True)
    in_maps = make_in_maps(**inputs, s_steps=s_steps)
    t1 = time.time()
    res = run_bass_kernel_spmd(nc, in_maps, list(range(NCORES)))
    print(f"neff+run: {time.time()-t1:.1f}s", flush=True)
    got = assemble_output(res.results, s_steps)
    want = np_ref(inputs, s_steps)
    rel = np.abs(got - want).max() / max(np.abs(want).max(), 1e-9)
    print(f"S={s_steps}  max|want|={np.abs(want).max():.4f}  rel_err={rel:.3e}", flush=True)


def build_timed_runner(nc, in_maps):
    """Device-resident executor for timing: stages inputs once, returns a
    callable that runs the NEFF across the 8 cores and blocks."""
    import jax
    import numpy as np
    from jax.sharding import Mesh, PartitionSpec, NamedSharding
    from jax.experimental.shard_map import shard_map
    from concourse import bass2jax, mybir as _mb
    from concourse.bass2jax import _bass_exec_p, partition_id_tensor, install_neuronx_cc_hook

    install_neuronx_cc_hook()
    n_cores = len(in_maps)
    part_name = nc.partition_id_tensor.name if nc.partition_id_tensor else None
    in_names, out_names, out_avals, zero_outs = [], [], [], []
    for alloc in nc.m.functions[0].allocations:
        if not isinstance(alloc, _mb.MemoryLocationSet):
            continue
        name = alloc.memorylocations[0].name
        if alloc.kind == "ExternalInput":
            if name != part_name:
                in_names.append(name)
        elif alloc.kind == "ExternalOutput":
            out_names.append(name)
            shape = tuple(alloc.tensor_shape)
            dtype = _mb.dt.np(alloc.dtype)
            out_avals.append(jax.core.ShapedArray(shape, dtype))
            zero_outs.append(np.zeros(shape, dtype))
    n_params = len(in_names)
    all_names = in_names + out_names
    if part_name is not None:
        all_names = all_names + [part_name]

    def _body(*args):
        operands = list(args)
        if part_name is not None:
            operands.append(partition_id_tensor())
        outs = _bass_exec_p.bind(
            *operands,
            out_avals=tuple(out_avals),
            in_names=tuple(all_names),
            out_names=tuple(out_names),
            lowering_input_output_aliases=(),
            sim_require_finite=True,
            sim_require_nnan=True,
            nc=nc,
        )
        return tuple(outs)

    devices = jax.devices()[:n_cores]
    mesh = Mesh(np.asarray(devices), ("core",))
    in_specs = (PartitionSpec("core"),) * (n_params + len(out_names))
    out_specs = (PartitionSpec("core"),) * len(out_names)
    sharded = jax.jit(
        shard_map(_body, mesh=mesh, in_specs=in_specs, out_specs=out_specs, check_rep=False),
        keep_unused=True,
    )
    sh = NamedSharding(mesh, PartitionSpec("core"))
    concat_in = [
        jax.device_put(
            np.concatenate(
                [np.asarray(in_maps[c][k]) for c in range(n_cores)], axis=0
            ),
            sh,
        )
        for k in in_names
    ]
    concat_zeros = [
        jax.device_put(np.zeros((n_cores * z.shape[0], *z.shape[1:]), z.dtype), sh)
        for z in zero_outs
    ]

    def run():
        outs = sharded(*concat_in, *concat_zeros)
        jax.block_until_ready(outs)
        return outs

    return run

